# revision 30
# baseline (speedup 1.0000x reference)
"""Multi-head attention (B=2, S=2048, D=2048, H=16, causal) on 8 TRN2 cores.

Sharding: heads are tensor-parallel end-to-end (2 heads per core). Each core
computes its 2 heads' attention AND its partial out-projection over the full
output (contraction over the local 256 head-dims only); the 8 partial outputs
are summed on the host. No collectives at all.

Everything is computed transposed: q/k are stored [d_head, seq] (as fp8),
scores are [keys, sq], attention output is [d_head, sq], partials are
yp [D, seq-block]. Softmax denominators are a ones-row matmul on the PE
over DVE-pre-summed groups of 4 P blocks.

fp8 (e4m3) with DoubleRow perf mode (0.5 PE cycles/row) carries the q/k
projections AND the score matmuls:
 - q/k: fp8(x) @ fp8(32 Wq/k) in [128,2]-packed DR (256 contraction rows
   per matmul = 4x bf16 throughput). The 32x weight pre-scale keeps the
   uniform(-1/sqrt(D),..) weights out of e4m3's subnormal range; 1/32^2
   is folded into the EXP activation's scale.
 - scores: dk=128 as [64,2] DR subtiles (q/k repacked on-chip via DMA).
The v projection, PV and out-projection matmuls stay bf16: fp8 in the
value path costs 2-5% rel error vs the 2e-2 budget, and the measured
3-term hi-lo fp8 v variant (V_V_FP8=1, subnormal-heavy residual
operands) ran ~120us SLOWER on silicon despite a lower cost-model time.

The causal mask is applied ON THE PE: each diagonal score block gets a
stack of 3 accumulating DR matmuls (240*I).T @ (-240*mask8) in e4m3,
planting -3*240^2 on masked entries so exp() yields ~3e-7 — no DVE mask
multiplies, and no e4m3<->e5m2 dtype switches in the PE stream.

Engine placement (sim-guided): phase 2 is elementwise-bound, so the
PSUM->bf16 yp copies are split (default 3:1) between DVE and Act — the
'exp_and_others' activation table contains Identity, so interleaving exp
and copy on Act costs no table reloads. Output DMAs ride the SP queue;
weights and x8/x8lo tiles ride the Act queue.

Softmax skips max-subtraction: scores/sqrt(dk) have std ~1/3, so exp()
cannot overflow. Matmuls accumulate in f32 PSUM.
"""

import sys

if "/opt/trn_rl_repo" not in sys.path:
    sys.path.insert(0, "/opt/trn_rl_repo")

import numpy as np
import ml_dtypes

import concourse.mybir as mybir
import concourse.tile as tile
from concourse import bacc
from concourse.bass_utils import run_bass_kernel_spmd

D = 2048          # model dim
H = 16            # heads
DK = 128          # head dim
B = 2             # batch
S = 2048          # seq per batch
SEQ = B * S       # flattened batch*seq = 4096
NCORES = 8
HPC = H // NCORES         # 2 heads per core
MC = HPC * DK             # 256 head-dims per core
KT = D // 128             # 16 contraction blocks
KT2 = KT // 2             # 8 DoubleRow contraction blocks (256 rows each)
ST = SEQ // 512           # 8 projection s-tiles
G = S // 512              # 4 sq-groups per batch
KBMAX = S // 128          # 16 key blocks per batch
BF = mybir.dt.bfloat16
F8 = mybir.dt.float8e4
F8E5 = mybir.dt.float8e5
F32 = mybir.dt.float32
EXP = mybir.ActivationFunctionType.Exp
IDENT = mybir.ActivationFunctionType.Identity
DR = mybir.MatmulPerfMode.DoubleRow
W8S = 32.0                # fp8 weight pre-scale
MASKV = 240.0             # e4m3 max; 3 stacked matmuls plant -3*MASKV^2
NMASK = 3                 # exp then sees score - 14.9 -> ~3e-7
ISCALE = float(1.0 / np.sqrt(DK))
ESCALE = float(ISCALE / (W8S * W8S))   # exp scale: 1/sqrt(dk) / 32^2

import os
NO_OUT_DMA = os.environ.get("V_NO_OUT_DMA", "0") == "1"  # skip yp writes (bench only)
QK_XLO = os.environ.get("V_QK_XLO", "")                  # "", "q", "k", "qk": add the
# x8lo @ W8 correction matmuls to those projections (~14us PE each,
# rel err 1.74e-2 -> 1.44e-2 per the numpy model)
V_FP8 = os.environ.get("V_V_FP8", "0") == "1"            # 3-term fp8 v projection;
# 0 falls back to the bf16 x/Wv path (needs the xT/wvT inputs)
PE_MASK = os.environ.get("V_PE_MASK", "1") == "1"        # causal mask on the PE;
# 0 falls back to DVE mask multiplies (needs the cmask input)
OUT_DMA_Q = os.environ.get("V_OUT_DMA_Q", "sync")        # engine queue for yp writes
MASK_ENG = os.environ.get("V_MASK_ENG", "vector")        # engine for fallback masks
YT_ENG = os.environ.get("V_YT_ENG", "split4")            # psum->bf16 yp copies:
# "vector" (all DVE), "scalar" (all Act), "splitN" (1 of N tiles on Act)
PRESUM = os.environ.get("V_PRESUM", "1") == "1"          # pre-sum P quads on DVE
# before the denominator matmuls; 0 = ones-matmul every P block on the PE

# In-NEFF repetition count for benchmarking (see bench.py); 1 for grading.
REPEATS = 1


def _build(repeats=1):
    nc = bacc.Bacc(None, num_devices=NCORES)
    x8T = nc.dram_tensor("x8", [ST, 128, 2, KT2, 512], F8, kind="ExternalInput")
    x8loT = (nc.dram_tensor("x8lo", [ST, 128, 2, KT2, 512], F8, kind="ExternalInput")
             if (V_FP8 or QK_XLO) else None)
    w8qT = nc.dram_tensor("w8q", [128, 2, KT2, MC], F8, kind="ExternalInput")
    w8kT = nc.dram_tensor("w8k", [128, 2, KT2, MC], F8, kind="ExternalInput")
    wv8T = (nc.dram_tensor("wv8", [128, 2, KT2, MC], F8, kind="ExternalInput")
            if V_FP8 else None)
    wv8loT = (nc.dram_tensor("wv8lo", [128, 2, KT2, MC], F8, kind="ExternalInput")
              if V_FP8 else None)
    xT = (nc.dram_tensor("xT", [ST, 128, KT, 512], BF, kind="ExternalInput")
          if not V_FP8 else None)
    wvT = (nc.dram_tensor("wvT", [128, KT, MC], BF, kind="ExternalInput")
           if not V_FP8 else None)
    woT = nc.dram_tensor("woT", [128, HPC, D], BF, kind="ExternalInput")
    bqk = nc.dram_tensor("bqk", [128, 4], F32, kind="ExternalInput")
    bvb = nc.dram_tensor("bvb", [128, MC], F32, kind="ExternalInput")
    cmask = (nc.dram_tensor("cmask", [128, 4, 512], BF, kind="ExternalInput")
             if not PE_MASK else None)
    cm8T = (nc.dram_tensor("cm8", [64, 2, 4, 512], F8, kind="ExternalInput")
            if PE_MASK else None)
    id8T = (nc.dram_tensor("id8", [64, 2, 128], F8, kind="ExternalInput")
            if PE_MASK else None)
    yp = nc.dram_tensor("yp", [KT, B, G, 128, 512], BF, kind="ExternalOutput")

    with tile.TileContext(nc) as tc:
        with (
            tc.tile_pool(name="const", bufs=1) as constp,
            tc.tile_pool(name="qkv", bufs=1) as qkvp,
        ):
            bqk_sb = constp.tile([128, 4], F32)
            nc.scalar.dma_start(bqk_sb[:], bqk.ap())
            bvb_sb = constp.tile([128, MC], F32)
            nc.scalar.dma_start(bvb_sb[:], bvb.ap())
            # phase-2-only constants: loads issued inside _body, after the
            # first projection weights/tiles, so they don't delay phase 1
            woT_sb = constp.tile([128, HPC, D], BF)
            if PE_MASK:
                cm8_sb = constp.tile([64, 2, 4, 512], F8)
                id8_sb = constp.tile([64, 2, 128], F8)
                cm_sb = None
            else:
                cm_sb = constp.tile([128, 4, 512], BF)
                cm8_sb = id8_sb = None
            ones_sb = constp.tile([128, 1], BF)
            nc.vector.memset(ones_sb[:], 1.0)

            for rep in range(repeats):
                _body(nc, tc, qkvp, (x8T, x8loT, xT), (w8qT, w8kT),
                      (wv8T, wv8loT, wvT), yp, woT_sb,
                      (cm8_sb, id8_sb, cm_sb), bqk_sb, bvb_sb, ones_sb,
                      load_consts=(rep == 0),
                      dram_consts=(woT, cm8T, id8T, cmask))

    nc.compile()
    return nc


def _body(nc, tc, qkvp, xTs, wqkTs, wvTs, yp, woT_sb, masks, bqk_sb, bvb_sb,
          ones_sb, load_consts=False, dram_consts=None):
    x8T, x8loT, xT = xTs
    w8qT, w8kT = wqkTs
    wv8T, wv8loT, wvT = wvTs
    cm8_sb, id8_sb, cm_sb = masks
    woT, cm8T, id8T, cmask = dram_consts
    need_xlo = V_FP8 or QK_XLO

    # persistent intermediates: q8d/k8d [64, 2, hl, seq] fp8 (dk = j*64 + p),
    # v [seq, vd] bf16 (32x scaled), staging q8s/k8s [128, hl, seq] fp8
    q8d = qkvp.tile([64, 2, HPC, SEQ], F8, tag="q8d")
    k8d = qkvp.tile([64, 2, HPC, SEQ], F8, tag="k8d")
    q8s = qkvp.tile([128, HPC, SEQ], F8, tag="q8s")
    k8s = qkvp.tile([128, HPC, SEQ], F8, tag="k8s")
    v_sb = qkvp.tile([128, SEQ // 128, MC], BF, tag="v_sb")

    # PE p-state warmup: the array ramps to full clock only after ~3us of
    # continuous work; these 1-row matmuls on the resident ones tile keep it
    # busy while the first weight/x DMAs land
    if load_consts:
        with tc.tile_pool(name="warm", bufs=1, space="PSUM") as warmp:
            wps = warmp.tile([1, 64], F32, tag="wps")
            for i in range(64):
                nc.tensor.matmul(wps[:, i:i + 1], ones_sb[:, 0:1], ones_sb[:, 0:1],
                                 start=(i == 0), stop=(i == 63))

    # ---- phase 1: QKV projections ----
    with (
        tc.tile_pool(name="w1", bufs=1) as w1p,
        tc.tile_pool(name="xt", bufs=2) as xtp,
        tc.tile_pool(name="ps1", bufs=2, space="PSUM") as ps1,
        tc.tile_pool(name="psv", bufs=2, space="PSUM") as psv,
    ):
        w8q_sb = w1p.tile([128, 2, KT2, MC], F8, tag="w8q")
        w8k_sb = w1p.tile([128, 2, KT2, MC], F8, tag="w8k")
        # q/k fp8 weights first (small, unblock the first matmuls), then the
        # first x8/x8lo tiles, then the v weights and phase-2 consts — all on
        # the scalar queue.
        nc.scalar.dma_start(w8q_sb[:], w8qT.ap())
        nc.scalar.dma_start(w8k_sb[:], w8kT.ap())
        if V_FP8:
            wv8_sb = w1p.tile([128, 2, KT2, MC], F8, tag="wv8")
            wv8lo_sb = w1p.tile([128, 2, KT2, MC], F8, tag="wv8lo")
        else:
            wv_sb = w1p.tile([128, KT, MC], BF, tag="wv")

        for st in range(ST):
            x8t = xtp.tile([128, 2, KT2, 512], F8, tag="x8t")
            nc.scalar.dma_start(x8t[:], x8T.ap()[st])
            if need_xlo:
                x8lo = xtp.tile([128, 2, KT2, 512], F8, tag="x8lo")
                nc.scalar.dma_start(x8lo[:], x8loT.ap()[st])
            if not V_FP8:
                xt = xtp.tile([128, KT, 512], BF, tag="xt")
                nc.sync.dma_start(xt[:], xT.ap()[st])
            if st == 0:
                if V_FP8:
                    nc.scalar.dma_start(wv8_sb[:], wv8T.ap())
                    nc.scalar.dma_start(wv8lo_sb[:], wv8loT.ap())
                else:
                    nc.scalar.dma_start(wv_sb[:], wvT.ap())
                if load_consts:
                    nc.scalar.dma_start(woT_sb[:], woT.ap())
                    if PE_MASK:
                        nc.scalar.dma_start(cm8_sb[:], cm8T.ap())
                        nc.scalar.dma_start(id8_sb[:], id8T.ap())
                    else:
                        nc.scalar.dma_start(cm_sb[:], cmask.ap())
            ssl = slice(st * 512, (st + 1) * 512)
            for hl in range(HPC):
                for w_sb, dst, bcol, xlo in ((w8q_sb, q8s, hl, "q" in QK_XLO),
                                             (w8k_sb, k8s, 2 + hl, "k" in QK_XLO)):
                    terms = [x8t] + ([x8lo] if xlo else [])
                    ps = ps1.tile([128, 512], F32, tag="ps1")
                    nmm = len(terms) * KT2
                    for ti, xop in enumerate(terms):
                        for kk in range(KT2):
                            i = ti * KT2 + kk
                            nc.tensor.matmul(ps[:], w_sb[:, :, kk, hl * 128:(hl + 1) * 128],
                                             xop[:, :, kk, :], start=(i == 0),
                                             stop=(i == nmm - 1), perf_mode=DR)
                    nc.scalar.activation(dst[:, hl, ssl], ps[:], IDENT,
                                         bias=bqk_sb[:, bcol:bcol + 1])
            for ss in range(4):
                pv = psv.tile([128, MC], F32, tag="psv")
                if V_FP8:
                    vterms = ((x8t, wv8_sb), (x8lo, wv8_sb), (x8t, wv8lo_sb))
                    nmm = len(vterms) * KT2
                    for ti, (xop, wop) in enumerate(vterms):
                        for kk in range(KT2):
                            i = ti * KT2 + kk
                            nc.tensor.matmul(pv[:], xop[:, :, kk, ss * 128:(ss + 1) * 128],
                                             wop[:, :, kk, :], start=(i == 0),
                                             stop=(i == nmm - 1), perf_mode=DR)
                else:
                    for k in range(KT):
                        nc.tensor.matmul(pv[:], xt[:, k, ss * 128:(ss + 1) * 128],
                                         wv_sb[:, k, :], start=(k == 0), stop=(k == KT - 1))
                nc.vector.tensor_add(v_sb[:, st * 4 + ss, :], pv[:], bvb_sb[:])

            # repack q/k fp8 into DoubleRow layout (subtile j <- partitions
            # 64j..) in two halves: batch 0's columns repack while the s-tiles
            # of batch 1 are still projecting, so phase 2 starts immediately
            if st in (ST // 2 - 1, ST - 1):
                csl = slice(0, 2048) if st == ST // 2 - 1 else slice(2048, SEQ)
                for stg, dst in ((q8s, q8d), (k8s, k8d)):
                    nc.scalar.dma_start(dst[:, 0, :, csl], stg[0:64, :, csl])
                    nc.scalar.dma_start(dst[:, 1, :, csl], stg[64:128, :, csl])

    # ---- phase 2: attention + partial out-projection per (batch, sq-group) --
    with (
        tc.tile_pool(name="pss", bufs=3, space="PSUM") as pss,
        tc.tile_pool(name="pso", bufs=2, space="PSUM") as pso,
        tc.tile_pool(name="pssum", bufs=1, space="PSUM") as pssum,
        tc.tile_pool(name="psy", bufs=2, space="PSUM") as psy,
        tc.tile_pool(name="aw", bufs=4) as aw,
    ):
        def outproj(b, g, atts):
            # partial out-projection for (b, g): contraction over the 256
            # local head-dims, all 2048 output features. Emitted one group
            # late so the att chain (PV -> recip -> broadcast -> mul) is
            # ready by the time the PE reaches these matmuls.
            for nt in range(KT):
                py = psy.tile([128, 512], F32, tag="py")
                nc.tensor.matmul(py[:], woT_sb[:, 0, nt * 128:(nt + 1) * 128],
                                 atts[0][:], start=True, stop=False)
                nc.tensor.matmul(py[:], woT_sb[:, 1, nt * 128:(nt + 1) * 128],
                                 atts[1][:], start=False, stop=True)
                yt = aw.tile([128, 512], BF, tag="yt", bufs=6)
                eng = YT_ENG
                if eng.startswith("split"):
                    n = int(eng[5:] or 2)
                    eng = "scalar" if nt % n == n - 1 else "vector"
                if eng == "scalar":
                    nc.scalar.activation(yt[:], py[:], IDENT)
                else:
                    getattr(nc, eng).tensor_copy(yt[:], py[:])
                if not NO_OUT_DMA:
                    eng = getattr(nc, OUT_DMA_Q)
                    eng.dma_start(yp.ap()[nt, b, g], yt[:])

        prev = None
        for b in range(B):
            for g in range(G):
                kb_max = 4 * (g + 1)
                # scores + exp + 4-block pre-sums for BOTH heads first, so
                # the denominator matmuls never stall the PE on the DVE adds
                per_hl = []
                for hl in range(HPC):
                    # For diagonal key-blocks (o = kb-4g >= 0) only sq >=
                    # 128*o is unmasked; compute just that slice.
                    Ps = []
                    for kb in range(kb_max):
                        o = kb - 4 * g
                        c0 = max(o, 0) * 128          # first valid sq column
                        qsl = slice(b * S + g * 512 + c0, b * S + (g + 1) * 512)
                        ksl = slice(b * S + kb * 128, b * S + (kb + 1) * 128)
                        ps = pss.tile([128, 512], F32, tag="pss")
                        diag = o >= 0 and PE_MASK
                        nc.tensor.matmul(ps[:, c0:], k8d[:, :, hl, ksl],
                                         q8d[:, :, hl, qsl],
                                         start=True, stop=not diag, perf_mode=DR)
                        if diag:
                            # plant -3*240^2 on masked entries via NMASK
                            # stacked accumulating matmuls (240 I).T @
                            # (-240 mask8), all e4m3 — mixing e5m2 into the
                            # e4m3 matmul stream costs ~1.5us per dtype
                            # switch on silicon. exp sees score-14.9 -> ~3e-7.
                            for mi in range(NMASK):
                                nc.tensor.matmul(ps[:, c0:], id8_sb[:],
                                                 cm8_sb[:, :, o, c0:],
                                                 start=False, stop=(mi == NMASK - 1),
                                                 perf_mode=DR)
                        P = aw.tile([128, 512], BF, tag="P", bufs=36)
                        nc.scalar.activation(P[:, c0:], ps[:, c0:], EXP, scale=ESCALE)
                        if o >= 0 and not PE_MASK:
                            getattr(nc, MASK_ENG).tensor_mul(
                                P[:, c0:], P[:, c0:], cm_sb[:, o, c0:])
                        Ps.append((P, c0))
                    # pre-sum groups of 4 P blocks for the denominator.
                    # off-diagonal groups: 3 full-width adds. diagonal group
                    # (the last one): staggered ranges — columns below each
                    # block's c0 were never written, and the group sum over
                    # those columns only involves earlier blocks.
                    P4s = []
                    for gi in range(kb_max // 4 if PRESUM else 0):
                        quad = Ps[4 * gi:4 * gi + 4]
                        P4 = aw.tile([128, 512], BF, tag="P4", bufs=10)
                        if quad[3][1] == 0:           # off-diagonal group
                            nc.vector.tensor_add(P4[:], quad[0][0][:], quad[1][0][:])
                            nc.vector.tensor_add(P4[:], P4[:], quad[2][0][:])
                            nc.vector.tensor_add(P4[:], P4[:], quad[3][0][:])
                        else:                          # diagonal group
                            nc.vector.tensor_copy(P4[:, :128], quad[0][0][:, :128])
                            nc.vector.tensor_add(P4[:, 128:], quad[0][0][:, 128:],
                                                 quad[1][0][:, 128:])
                            nc.vector.tensor_add(P4[:, 256:], P4[:, 256:],
                                                 quad[2][0][:, 256:])
                            nc.vector.tensor_add(P4[:, 384:], P4[:, 384:],
                                                 quad[3][0][:, 384:])
                        P4s.append(P4)
                    per_hl.append((Ps, P4s))

                atts = []
                for hl in range(HPC):
                    Ps, P4s = per_hl[hl]
                    psm = pssum.tile([1, 512], F32, tag="psm")
                    if PRESUM:
                        for gi, P4 in enumerate(P4s):
                            nc.tensor.matmul(psm[:], ones_sb[:], P4[:],
                                             start=(gi == 0), stop=(gi == len(P4s) - 1))
                    else:
                        for kb, (P, c0) in enumerate(Ps):
                            nc.tensor.matmul(psm[:, c0:], ones_sb[:], P[:, c0:],
                                             start=(kb == 0), stop=(kb == kb_max - 1))
                    po = pso.tile([128, 512], F32, tag="po")
                    for kb, (P, c0) in enumerate(Ps):
                        nc.tensor.matmul(po[:, c0:],
                                         v_sb[:, b * 16 + kb, hl * 128:(hl + 1) * 128],
                                         P[:, c0:], start=(kb == 0), stop=(kb == kb_max - 1))
                    recip = aw.tile([1, 512], F32, tag="recip")
                    nc.vector.reciprocal(recip[:], psm[:])
                    rb = aw.tile([128, 512], F32, tag="rb")
                    nc.gpsimd.partition_broadcast(rb[:], recip[:])
                    att = aw.tile([128, 512], BF, tag="att", bufs=6)
                    nc.vector.tensor_mul(att[:], po[:], rb[:])
                    atts.append(att)
                if prev is not None:
                    outproj(*prev)
                prev = (b, g, atts)
        outproj(*prev)


def _prep_inputs(x, Wq, bq, Wk, bk, Wv, bv, Wo, bo):
    bf16 = ml_dtypes.bfloat16
    f8 = mybir.dt.np(F8)
    f32 = np.float32

    xf = np.ascontiguousarray(x.reshape(SEQ, D).T)            # [D, SEQ]
    xT_t = np.ascontiguousarray(
        xf.reshape(KT, 128, ST, 512).transpose(2, 1, 0, 3)).astype(bf16)
    # DoubleRow layout: [ST, 128, 2, KT2, 512], pairing D-blocks (2kk, 2kk+1)
    xd = np.ascontiguousarray(
        xf.reshape(KT2, 2, 128, ST, 512).transpose(3, 2, 1, 0, 4))
    x8_t = xd.astype(f8)
    x8lo_t = (xd - x8_t.astype(np.float32)).astype(f8)

    o_idx = np.arange(4)[None, :, None]
    p_idx = np.arange(128)[:, None, None]
    s_idx = np.arange(512)[None, None, :]
    cmask = (p_idx + 128 * o_idx <= s_idx).astype(bf16)       # [128, 4, 512]
    # PE-mask tensors: d = 64*j + p indexes the key within the block
    d_idx = (64 * np.arange(2)[None, :, None] + np.arange(64)[:, None, None])
    cm8 = np.where(d_idx[:, :, :, None] + 128 * np.arange(4)[None, None, :, None]
                   > s_idx[None], -MASKV, 0.0).astype(f8)     # [64, 2, 4, 512]
    id8 = np.zeros((64, 2, 128), np.float32)
    pj = np.arange(128)
    id8[pj % 64, pj // 64, pj] = MASKV
    id8 = id8.astype(f8)

    in_maps = []
    for c in range(NCORES):
        hs = slice(c * MC, (c + 1) * MC)

        def wt8(w):
            # 32x-scaled fp8 weights + fp8 of the quantization residual,
            # in DoubleRow layout [128, 2, KT2, MC]
            wc = np.ascontiguousarray(w[hs, :].T) * W8S        # [D, MC] scaled
            wd = np.ascontiguousarray(
                wc.reshape(KT2, 2, 128, MC).transpose(2, 1, 0, 3))
            hi = wd.astype(f8)
            lo = (wd - hi.astype(np.float32)).astype(f8)
            return hi, lo

        def wt(w):
            wc = np.ascontiguousarray(w[hs, :].T)              # [D, MC]
            return np.ascontiguousarray(
                wc.reshape(KT, 128, MC).transpose(1, 0, 2)).astype(bf16)

        woT_c = np.ascontiguousarray(
            Wo[:, hs].T.reshape(HPC, 128, D).transpose(1, 0, 2)).astype(bf16)
        bq_c = (bq[hs] * W8S).astype(f32)
        bk_c = (bk[hs] * W8S).astype(f32)
        bqk_c = np.stack([bq_c[:128], bq_c[128:], bk_c[:128], bk_c[128:]], axis=1)
        vbias = bv[hs] * (W8S if V_FP8 else 1.0)
        bvb_c = np.ascontiguousarray(np.broadcast_to(vbias, (128, MC))).astype(f32)
        wv8_hi, wv8_lo = wt8(Wv)
        in_maps.append({
            "x8": x8_t, "x8lo": x8lo_t, "xT": xT_t,
            "w8q": wt8(Wq)[0], "w8k": wt8(Wk)[0],
            "wv8": wv8_hi, "wv8lo": wv8_lo, "wvT": wt(Wv),
            "woT": woT_c, "bqk": bqk_c, "bvb": bvb_c,
            "cmask": cmask, "cm8": cm8, "id8": id8,
        })
    return in_maps


_NC_CACHE = {}


def kernel(x, Wq, bq, Wk, bk, Wv, bv, Wo, bo):
    args = [np.asarray(a, np.float32) for a in (x, Wq, bq, Wk, bk, Wv, bv, Wo, bo)]
    in_maps = _prep_inputs(*args)
    if REPEATS not in _NC_CACHE:
        _NC_CACHE[REPEATS] = _build(REPEATS)
    nc = _NC_CACHE[REPEATS]
    r = run_bass_kernel_spmd(nc, in_maps, core_ids=list(range(NCORES)))
    acc = np.zeros((KT, B, G, 128, 512), np.float32)
    for c in range(NCORES):
        acc += r.results[c]["yp"].astype(np.float32)
    if V_FP8:
        acc /= W8S                                            # v path runs 32x hot
    y = acc.transpose(0, 3, 1, 2, 4).reshape(D, SEQ)          # [n, b*S + g*512 + s]
    y += np.asarray(bo, np.float32)[:, None]
    return np.ascontiguousarray(y.T).reshape(B, S, D).astype(np.float32)
